# revision 15
# baseline (speedup 1.0000x reference)
"""ConvLSTM cell (B=32, C_IN=32, HC=64, H=W=64, K=3) on 8 trn2 NeuronCores.

Strategy: data-parallel over batch (4 images per core), weights replicated.

The 3x3 conv over 96 input channels (x:32 + h:64) is restructured so every
matmul contracts a full 128-partition column: the 9 taps x 96 channels = 864
contraction rows are packed into 7 passes of <=128 rows. Each block loads
three SBUF tiles whose partition groups hold shift-baked copies of the
(zero-padded) input window:
  T1[128] = [h base        | h shifted (0,+1)]          passes 0-2 (dy=0,1,2)
  T2[128] = [h shifted (0,+2) | x base | x shifted (0,+1)]  passes 3-5
  T3[96]  = [x (0,+2) | x (+1,+2) | x (+2,+2)]          pass 6
Reading a tile at row-offset dy gives taps (dy, dx) for every group at once,
so one matmul covers 2-3 taps; 7 passes replace the naive 9 and use the PE
array's full 128 contraction rows. Zero padding lives in HBM (host pads to
[96, 68, 66]), so there are no border-restricted access patterns.

Gate columns: chunk0 = [f, i], chunk1 = [o, g]. PSUM accumulates over the 7
passes; the LSTM elementwise math runs per block with one cross-partition
copy, and h_new/c_new are written back as bf16 in a single merged DMA.
"""

import os
import sys

import numpy as np

if "/opt/trn_rl_repo" not in sys.path:
    sys.path.insert(0, "/opt/trn_rl_repo")

import ml_dtypes

BF16 = ml_dtypes.bfloat16

B, C_IN, HC, H, W, K = 32, 32, 64, 64, 64, 3
N_CORES = 8
B_LOC = B // N_CORES  # 4 images per core
CTOT = C_IN + HC  # 96 combined input channels
HP, WP = H + 4, W + 2  # padded image: rows 1..64 real, cols 1..64 real
RPB = 16  # output rows per block (8 for the last image)
SUB_ROWS = 8  # rows per matmul (512 px = one PSUM bank)
# per-pass (tile, dy) schedule: passes 0-2 on T1, 3-5 on T2, 6 on T3
PASS_TILE = [0, 0, 0, 1, 1, 1, 2]
PASS_DY = [0, 1, 2, 0, 1, 2, 0]
# pass 6 carries a 97th all-ones row whose weights are the gate biases
PASS_ROWS = [128, 128, 128, 128, 128, 128, 97]
NPASS = 7

_CACHE: dict = {}


def _build_program():
    import concourse.bacc as bacc
    import concourse.mybir as mybir
    import concourse.tile as tile

    nc = bacc.Bacc("TRN2", target_bir_lowering=False, debug=False)
    f32 = mybir.dt.float32
    bf16 = mybir.dt.bfloat16
    AF = mybir.ActivationFunctionType

    # host-baked shift tiles: each [B_LOC, ch, HP, W] with fully contiguous
    # per-partition windows (one DMA per tile per block)
    t1_d = nc.dram_tensor("t1", [B_LOC, 128, HP, W], bf16, kind="ExternalInput").ap()
    t2_d = nc.dram_tensor("t2", [B_LOC, 128, HP, W], bf16, kind="ExternalInput").ap()
    t3_d = nc.dram_tensor("t3", [B_LOC, 97, HP, W], bf16, kind="ExternalInput").ap()
    c_d = nc.dram_tensor("c", [B_LOC, HC, H, W], bf16, kind="ExternalInput").ap()
    # packed weights: 7 passes x (chunk0 128 cols | chunk1 128 cols)
    w_d = nc.dram_tensor("w", [128, NPASS * 256], bf16, kind="ExternalInput").ap()
    # out[:, 0] = h_new, out[:, 1] = c_new  (bf16; host upcasts)
    out_d = nc.dram_tensor(
        "out", [B_LOC, 2, HC, H, W], bf16, kind="ExternalOutput"
    ).ap()

    with tile.TileContext(nc) as tc:
        with (
            tc.tile_pool(name="const", bufs=1) as constp,
            tc.tile_pool(name="pt", bufs=2) as ptp,
            tc.tile_pool(name="psum0", bufs=2, space="PSUM") as pp0,
            tc.tile_pool(name="psum1", bufs=2, space="PSUM") as pp1,
            tc.tile_pool(name="work", bufs=3) as sp,
        ):
            w_sb = constp.tile([128, NPASS * 256], bf16)
            nc.scalar.dma_start(w_sb[:], w_d)

            # PE prewarm: dummy matmuls on zeroed tiles open the HAM clock
            # gate while the first input DMAs are in flight (one accumulation
            # group so the scheduler can't drop them as dead writes)
            dw = constp.tile([128, 128], bf16)
            nc.gpsimd.memset(dw[:], 0.0)
            drh = constp.tile([128, 512], bf16)
            nc.gpsimd.memset(drh[:], 0.0)
            NPW = 5
            pwp = pp0.tile([128, RPB * W], f32, tag="P0", name="pw")
            for i in range(NPW):
                nc.tensor.matmul(
                    pwp[:, 0:512], dw[:], drh[:], start=(i == 0), stop=(i == NPW - 1)
                )

            def stage_b(st):
                # deferred tail of a block: tanh(c_new), h_new
                so_, chn_, px_, i_ = st
                tch = sp.tile([64, px_], f32, tag="tch", name=f"tch{i_}")
                nc.scalar.activation(tch[:], chn_[64:128, :], AF.Tanh)
                nc.vector.tensor_mul(chn_[0:64, :], so_[:], tch[:])

            bi = 0
            for b in range(B_LOC):
                rpb = 8 if b == B_LOC - 1 else RPB
                nblk = H // rpb

                # whole-image shift-baked input tiles (one DMA each; image 0
                # split so block 0 can start as soon as its rows land)
                t1 = ptp.tile([128, HP * W], bf16, tag="t1", name=f"t1_{b}")
                t2 = ptp.tile([128, HP * W], bf16, tag="t2", name=f"t2_{b}")
                t3 = ptp.tile([97, HP * W], bf16, tag="t3", name=f"t3_{b}")
                t1v = t1[:].rearrange("c (y x) -> c y x", x=W)
                t2v = t2[:].rearrange("c (y x) -> c y x", x=W)
                t3v = t3[:].rearrange("c (y x) -> c y x", x=W)
                splits = (0, 18, HP) if b == 0 else (0, HP)
                for r0_, r1_ in zip(splits[:-1], splits[1:]):
                    nc.sync.dma_start(t1v[:, r0_:r1_, :], t1_d[b, :, r0_:r1_, :])
                    nc.sync.dma_start(t2v[:, r0_:r1_, :], t2_d[b, :, r0_:r1_, :])
                    nc.gpsimd.dma_start(t3v[:, r0_:r1_, :], t3_d[b, :, r0_:r1_, :])
                tiles = (t1v, t2v, t3v)

                # whole-image [h_new | c_new] staging tile (bf16)
                ho = sp.tile([128, H * W], bf16, tag="ho", name=f"ho{b}", bufs=2)

                pending = None
                for blk in range(nblk):
                    y0 = blk * rpb
                    blk_px = rpb * W
                    sl = slice(y0 * W, y0 * W + blk_px)

                    P0 = pp0.tile([128, blk_px], f32, tag="P0", name=f"P0_{bi}")
                    P1 = pp1.tile([128, blk_px], f32, tag="P1", name=f"P1_{bi}")
                    for chunk, P in ((0, P0), (1, P1)):
                        P3 = P[:].rearrange("c (y x) -> c y x", x=W)
                        for p in range(NPASS):
                            tv = tiles[PASS_TILE[p]]
                            dy = PASS_DY[p]
                            rows = PASS_ROWS[p]
                            lo = p * 256 + chunk * 128
                            lhsT = w_sb[0:rows, lo : lo + 128]
                            for sub in range(rpb // SUB_ROWS):
                                r0 = y0 + sub * SUB_ROWS
                                rhs = tv[0:rows, r0 + dy : r0 + dy + SUB_ROWS, :]
                                nc.tensor.matmul(
                                    P3[:, sub * SUB_ROWS : sub * SUB_ROWS + SUB_ROWS, :],
                                    lhsT,
                                    rhs,
                                    start=(p == 0),
                                    stop=(p == NPASS - 1),
                                )

                    # elementwise LSTM math for this block
                    # P0 = [f | i], P1 = [o | g] (by 64-partition halves)
                    s_fi = sp.tile([128, blk_px], f32, tag="sfi", name=f"sfi{bi}")
                    nc.scalar.activation(s_fi[:], P0[:], AF.Sigmoid)
                    so = sp.tile([64, blk_px], f32, tag="so", name=f"so{bi}")
                    nc.scalar.activation(so[:], P1[0:64, :], AF.Sigmoid)
                    cg = sp.tile([128, blk_px], bf16, tag="cg", name=f"cg{bi}")
                    nc.scalar.activation(cg[64:128, :], P1[64:128, :], AF.Tanh)
                    nc.gpsimd.dma_start(
                        cg[0:64, :].rearrange("c (y x) -> c y x", x=W),
                        c_d[b, :, y0 : y0 + rpb, :],
                    )
                    # prd = [f*c | i*g]
                    prd = sp.tile([128, blk_px], f32, tag="prd", name=f"prd{bi}")
                    nc.vector.tensor_mul(prd[:], s_fi[:], cg[:])
                    igc = sp.tile([64, blk_px], f32, tag="igc", name=f"igc{bi}")
                    nc.vector.tensor_copy(igc[:], prd[64:128, :])
                    # c_new into the image staging tile
                    nc.vector.tensor_add(ho[64:128, sl], prd[0:64, :], igc[:])
                    if pending is not None:
                        stage_b(pending)
                    pending = (so, ho[:, sl], blk_px, bi)
                    bi += 1
                stage_b(pending)

                # one output DMA per image (split for the last image so the
                # final transfer overlaps the tail)
                osp = (0, H // 2, H) if b == B_LOC - 1 else (0, H)
                for r0_, r1_ in zip(osp[:-1], osp[1:]):
                    nc.sync.dma_start(
                        out_d[b, :, :, r0_:r1_, :].rearrange("t c y x -> (t c) y x"),
                        ho[:, r0_ * W : r1_ * W].rearrange("p (y x) -> p y x", x=W),
                    )

    nc.compile()
    return nc


def get_program():
    if "nc" not in _CACHE:
        _CACHE["nc"] = _build_program()
    return _CACHE["nc"]


def _prep_host(inputs):
    """Pack weights per pass; pad+concat x/h (bf16); per-core input maps."""
    x = np.asarray(inputs["x"], np.float32)
    h = np.asarray(inputs["hidden_state"], np.float32)
    c = np.asarray(inputs["cell_state"], np.float32).astype(BF16)

    # gate column order [f, i] (chunk0), [o, g] (chunk1)
    gx = [inputs["w_xf"], inputs["w_xi"], inputs["w_xo"], inputs["w_xg"]]
    gh = [inputs["w_hf"], inputs["w_hi"], inputs["w_ho"], inputs["w_hg"]]
    wx = np.stack([np.asarray(a, np.float32) for a in gx])  # [4, HC, C_IN, 3, 3]
    wh = np.stack([np.asarray(a, np.float32) for a in gh])  # [4, HC, HC, 3, 3]
    # -> [in_ch, dy, dx, 4*HC out]
    wxc = np.transpose(wx, (2, 3, 4, 0, 1)).reshape(C_IN, 3, 3, 4 * HC)
    whc = np.transpose(wh, (2, 3, 4, 0, 1)).reshape(HC, 3, 3, 4 * HC)

    # pass row packing must mirror the SBUF tile partition groups
    wpk = np.zeros((128, NPASS, 4 * HC), np.float32)
    for p in range(3):  # T1: h(dy,0) | h(dy,1)
        wpk[0:64, p] = whc[:, p, 0]
        wpk[64:128, p] = whc[:, p, 1]
    for p in range(3):  # T2: h(dy,2) | x(dy,0) | x(dy,1)
        wpk[0:64, 3 + p] = whc[:, p, 2]
        wpk[64:96, 3 + p] = wxc[:, p, 0]
        wpk[96:128, 3 + p] = wxc[:, p, 1]
    # T3: x(0,2) | x(1,2) | x(2,2) | ones row carrying the gate biases
    wpk[0:32, 6] = wxc[:, 0, 2]
    wpk[32:64, 6] = wxc[:, 1, 2]
    wpk[64:96, 6] = wxc[:, 2, 2]
    bf = np.asarray(inputs["b_xf"], np.float32) + np.asarray(inputs["b_hf"], np.float32)
    bi_ = np.asarray(inputs["b_xi"], np.float32) + np.asarray(inputs["b_hi"], np.float32)
    bo = np.asarray(inputs["b_xo"], np.float32) + np.asarray(inputs["b_ho"], np.float32)
    bg = np.asarray(inputs["b_xg"], np.float32) + np.asarray(inputs["b_hg"], np.float32)
    wpk[96, 6] = np.concatenate([bf, bi_, bo, bg])
    wcat = wpk.reshape(128, NPASS * 4 * HC).astype(BF16)

    # padded image [B, 96, HP+2, WP], h channels first; extra rows/cols give
    # room for the baked shifts below
    HPP, WPP = HP + 2, W + 2
    img = np.zeros((B, CTOT, HPP, WPP), np.float32)
    img[:, 0:HC, 1 : 1 + H, 1 : 1 + W] = h
    img[:, HC:CTOT, 1 : 1 + H, 1 : 1 + W] = x

    def shift(c0, c1, sy, sx):
        return img[:, c0:c1, sy : sy + HP, sx : sx + W]

    # bake shifts so each block's tile loads are a single contiguous window
    t1 = np.concatenate([shift(0, HC, 0, 0), shift(0, HC, 0, 1)], 1).astype(BF16)
    t2 = np.concatenate(
        [shift(0, HC, 0, 2), shift(HC, CTOT, 0, 0), shift(HC, CTOT, 0, 1)], 1
    ).astype(BF16)
    ones = np.ones((B, 1, HP, W), np.float32)
    t3 = np.concatenate(
        [shift(HC, CTOT, 0, 2), shift(HC, CTOT, 1, 2), shift(HC, CTOT, 2, 2), ones], 1
    ).astype(BF16)

    in_maps = []
    for i in range(N_CORES):
        s = slice(i * B_LOC, (i + 1) * B_LOC)
        in_maps.append(
            {
                "t1": np.ascontiguousarray(t1[s]),
                "t2": np.ascontiguousarray(t2[s]),
                "t3": np.ascontiguousarray(t3[s]),
                "c": c[s],
                "w": wcat,
            }
        )
    return in_maps


def run(inputs, trace=False, trace_kwargs=None):
    from concourse.bass_utils import run_bass_kernel_spmd

    nc = get_program()
    in_maps = _prep_host(inputs)
    res = run_bass_kernel_spmd(
        nc,
        in_maps,
        list(range(N_CORES)),
        trace=trace,
        **(trace_kwargs or {}),
    )
    h_new = np.concatenate([r["out"][:, 0] for r in res.results], 0).astype(
        np.float32
    )
    c_new = np.concatenate([r["out"][:, 1] for r in res.results], 0).astype(
        np.float32
    )
    return (h_new, c_new), res


def kernel(**inputs):
    (h_new, c_new), _ = run(inputs, trace=False)
    return (h_new, c_new)


# revision 19
# speedup vs baseline: 1.4812x; 1.4812x over previous
"""ConvLSTM cell (B=32, C_IN=32, HC=64, H=W=64, K=3) on 8 trn2 NeuronCores.

Strategy: data-parallel over batch (4 images per core), weights replicated.

The 3x3 conv over 96 input channels (x:32 + h:64) is restructured so every
matmul contracts a full 128-partition column: the 9 taps x 96 channels = 864
contraction rows are packed into 7 passes of <=128 rows. Each block loads
three SBUF tiles whose partition groups hold shift-baked copies of the
(zero-padded) input window:
  T1[128] = [h base        | h shifted (0,+1)]          passes 0-2 (dy=0,1,2)
  T2[128] = [h shifted (0,+2) | x base | x shifted (0,+1)]  passes 3-5
  T3[96]  = [x (0,+2) | x (+1,+2) | x (+2,+2)]          pass 6
Reading a tile at row-offset dy gives taps (dy, dx) for every group at once,
so one matmul covers 2-3 taps; 7 passes replace the naive 9 and use the PE
array's full 128 contraction rows. Zero padding lives in HBM (host pads to
[96, 68, 66]), so there are no border-restricted access patterns.

Gate columns: chunk0 = [f, i], chunk1 = [o, g]. PSUM accumulates over the 7
passes; the LSTM elementwise math runs per block with one cross-partition
copy, and h_new/c_new are written back as bf16 in a single merged DMA.
"""

import os
import sys

import numpy as np

if "/opt/trn_rl_repo" not in sys.path:
    sys.path.insert(0, "/opt/trn_rl_repo")

import ml_dtypes

BF16 = ml_dtypes.bfloat16

B, C_IN, HC, H, W, K = 32, 32, 64, 64, 64, 3
N_CORES = 8
B_LOC = B // N_CORES  # 4 images per core
CTOT = C_IN + HC  # 96 combined input channels
HP, WP = H + 4, W + 2  # padded image: rows 1..64 real, cols 1..64 real
RPB = 16  # output rows per block (8 for the last image)
SUB_ROWS = 8  # rows per matmul (512 px = one PSUM bank)
# per-pass (tile, dy) schedule: passes 0-2 on T1, 3-5 on T2, 6 on T3
PASS_TILE = [0, 0, 0, 1, 1, 1, 2]
PASS_DY = [0, 1, 2, 0, 1, 2, 0]
# pass 6 carries a 97th all-ones row whose weights are the gate biases
PASS_ROWS = [128, 128, 128, 128, 128, 128, 97]
NPASS = 7

_CACHE: dict = {}


def _build_program():
    import concourse.bacc as bacc
    import concourse.mybir as mybir
    import concourse.tile as tile

    nc = bacc.Bacc("TRN2", target_bir_lowering=False, debug=False)
    f32 = mybir.dt.float32
    bf16 = mybir.dt.bfloat16
    AF = mybir.ActivationFunctionType

    # host-baked shift tiles: each [B_LOC, ch, HP, W] with fully contiguous
    # per-partition windows (one DMA per tile per block)
    t1_d = nc.dram_tensor("t1", [B_LOC, 128, HP, W], bf16, kind="ExternalInput").ap()
    t2_d = nc.dram_tensor("t2", [B_LOC, 128, HP, W], bf16, kind="ExternalInput").ap()
    t3_d = nc.dram_tensor("t3", [B_LOC, 97, HP, W], bf16, kind="ExternalInput").ap()
    c_d = nc.dram_tensor("c", [B_LOC, HC, H, W], bf16, kind="ExternalInput").ap()
    # packed weights: 7 passes x (chunk0 128 cols | chunk1 128 cols)
    w_d = nc.dram_tensor("w", [128, NPASS * 256], bf16, kind="ExternalInput").ap()
    # out[:, 0] = h_new, out[:, 1] = c_new  (bf16; host upcasts)
    out_d = nc.dram_tensor(
        "out", [B_LOC, 2, HC, H, W], bf16, kind="ExternalOutput"
    ).ap()

    with tile.TileContext(nc) as tc:
        with (
            tc.tile_pool(name="const", bufs=1) as constp,
            tc.tile_pool(name="pt", bufs=2) as ptp,
            tc.tile_pool(name="psum0", bufs=2, space="PSUM") as pp0,
            tc.tile_pool(name="psum1", bufs=2, space="PSUM") as pp1,
            tc.tile_pool(name="work", bufs=3) as sp,
        ):
            w_sb = constp.tile([128, NPASS * 256], bf16)
            nc.scalar.dma_start(w_sb[:], w_d)

            # PE prewarm: dummy matmuls on zeroed tiles open the HAM clock
            # gate while the first input DMAs are in flight (one accumulation
            # group so the scheduler can't drop them as dead writes)
            dw = constp.tile([128, 128], bf16)
            nc.gpsimd.memset(dw[:], 0.0)
            drh = constp.tile([128, 512], bf16)
            nc.gpsimd.memset(drh[:], 0.0)
            NPW = 5
            pwp = pp0.tile([128, RPB * W], f32, tag="P0", name="pw")
            for i in range(NPW):
                nc.tensor.matmul(
                    pwp[:, 0:512], dw[:], drh[:], start=(i == 0), stop=(i == NPW - 1)
                )

            def stage_b(st):
                # deferred tail of a block: tanh(c_new), h_new
                so_, chn_, px_, i_ = st
                tch = sp.tile([64, px_], f32, tag="tch", name=f"tch{i_}")
                nc.scalar.activation(tch[:], chn_[64:128, :], AF.Tanh)
                nc.vector.tensor_mul(chn_[0:64, :], so_[:], tch[:])

            bi = 0
            for b in range(B_LOC):
                rpb = 8 if b == B_LOC - 1 else RPB
                nblk = H // rpb

                # whole-image shift-baked input tiles (one DMA each; image 0
                # split so block 0 can start as soon as its rows land)
                t1 = ptp.tile([128, HP * W], bf16, tag="t1", name=f"t1_{b}")
                t2 = ptp.tile([128, HP * W], bf16, tag="t2", name=f"t2_{b}")
                t3 = ptp.tile([97, HP * W], bf16, tag="t3", name=f"t3_{b}")
                t1v = t1[:].rearrange("c (y x) -> c y x", x=W)
                t2v = t2[:].rearrange("c (y x) -> c y x", x=W)
                t3v = t3[:].rearrange("c (y x) -> c y x", x=W)
                # split transfers across partition groups: each DMA queue
                # moves ~25 GB/s, so parallel queues are needed for bandwidth
                splits = (0, 18, HP) if b == 0 else (0, HP)
                for r0_, r1_ in zip(splits[:-1], splits[1:]):
                    for p0, p1 in ((0, 32), (32, 64), (64, 96), (96, 128)):
                        nc.sync.dma_start(
                            t1v[p0:p1, r0_:r1_, :], t1_d[b, p0:p1, r0_:r1_, :]
                        )
                        nc.sync.dma_start(
                            t2v[p0:p1, r0_:r1_, :], t2_d[b, p0:p1, r0_:r1_, :]
                        )
                    for p0, p1 in ((0, 32), (32, 64), (64, 97)):
                        nc.gpsimd.dma_start(
                            t3v[p0:p1, r0_:r1_, :], t3_d[b, p0:p1, r0_:r1_, :]
                        )
                tiles = (t1v, t2v, t3v)

                # whole-image [h_new | c_new] staging tile (bf16)
                ho = sp.tile([128, H * W], bf16, tag="ho", name=f"ho{b}", bufs=2)

                pending = None
                for blk in range(nblk):
                    y0 = blk * rpb
                    blk_px = rpb * W
                    sl = slice(y0 * W, y0 * W + blk_px)

                    P0 = pp0.tile([128, blk_px], f32, tag="P0", name=f"P0_{bi}")
                    P1 = pp1.tile([128, blk_px], f32, tag="P1", name=f"P1_{bi}")
                    for chunk, P in ((0, P0), (1, P1)):
                        P3 = P[:].rearrange("c (y x) -> c y x", x=W)
                        for p in range(NPASS):
                            tv = tiles[PASS_TILE[p]]
                            dy = PASS_DY[p]
                            rows = PASS_ROWS[p]
                            lo = p * 256 + chunk * 128
                            lhsT = w_sb[0:rows, lo : lo + 128]
                            for sub in range(rpb // SUB_ROWS):
                                r0 = y0 + sub * SUB_ROWS
                                rhs = tv[0:rows, r0 + dy : r0 + dy + SUB_ROWS, :]
                                nc.tensor.matmul(
                                    P3[:, sub * SUB_ROWS : sub * SUB_ROWS + SUB_ROWS, :],
                                    lhsT,
                                    rhs,
                                    start=(p == 0),
                                    stop=(p == NPASS - 1),
                                )

                    # elementwise LSTM math for this block
                    # P0 = [f | i], P1 = [o | g] (by 64-partition halves)
                    s_fi = sp.tile([128, blk_px], f32, tag="sfi", name=f"sfi{bi}")
                    nc.scalar.activation(s_fi[:], P0[:], AF.Sigmoid)
                    so = sp.tile([64, blk_px], f32, tag="so", name=f"so{bi}")
                    nc.scalar.activation(so[:], P1[0:64, :], AF.Sigmoid)
                    cg = sp.tile([128, blk_px], bf16, tag="cg", name=f"cg{bi}")
                    nc.scalar.activation(cg[64:128, :], P1[64:128, :], AF.Tanh)
                    nc.sync.dma_start(
                        cg[0:64, :].rearrange("c (y x) -> c y x", x=W),
                        c_d[b, :, y0 : y0 + rpb, :],
                    )
                    # prd = [f*c | i*g]
                    prd = sp.tile([128, blk_px], f32, tag="prd", name=f"prd{bi}")
                    nc.vector.tensor_mul(prd[:], s_fi[:], cg[:])
                    igc = sp.tile([64, blk_px], f32, tag="igc", name=f"igc{bi}")
                    nc.vector.tensor_copy(igc[:], prd[64:128, :])
                    # c_new into the image staging tile
                    nc.vector.tensor_add(ho[64:128, sl], prd[0:64, :], igc[:])
                    if pending is not None:
                        stage_b(pending)
                    pending = (so, ho[:, sl], blk_px, bi)
                    bi += 1
                stage_b(pending)

                # output DMAs per image on the scalar queue (its data deps are
                # satisfied just before, so it never stalls input loads); h and
                # c halves go to separate queues, last image also row-split
                osp = (0, H // 2, H) if b == B_LOC - 1 else (0, H)
                for r0_, r1_ in zip(osp[:-1], osp[1:]):
                    for t_, (p0, p1) in ((0, (0, 64)), (1, (64, 128))):
                        nc.scalar.dma_start(
                            out_d[b, t_, :, r0_:r1_, :],
                            ho[p0:p1, r0_ * W : r1_ * W].rearrange(
                                "p (y x) -> p y x", x=W
                            ),
                        )

    nc.compile()
    return nc


def get_program():
    if "nc" not in _CACHE:
        _CACHE["nc"] = _build_program()
    return _CACHE["nc"]


def _prep_host(inputs):
    """Pack weights per pass; pad+concat x/h (bf16); per-core input maps."""
    x = np.asarray(inputs["x"], np.float32)
    h = np.asarray(inputs["hidden_state"], np.float32)
    c = np.asarray(inputs["cell_state"], np.float32).astype(BF16)

    # gate column order [f, i] (chunk0), [o, g] (chunk1)
    gx = [inputs["w_xf"], inputs["w_xi"], inputs["w_xo"], inputs["w_xg"]]
    gh = [inputs["w_hf"], inputs["w_hi"], inputs["w_ho"], inputs["w_hg"]]
    wx = np.stack([np.asarray(a, np.float32) for a in gx])  # [4, HC, C_IN, 3, 3]
    wh = np.stack([np.asarray(a, np.float32) for a in gh])  # [4, HC, HC, 3, 3]
    # -> [in_ch, dy, dx, 4*HC out]
    wxc = np.transpose(wx, (2, 3, 4, 0, 1)).reshape(C_IN, 3, 3, 4 * HC)
    whc = np.transpose(wh, (2, 3, 4, 0, 1)).reshape(HC, 3, 3, 4 * HC)

    # pass row packing must mirror the SBUF tile partition groups
    wpk = np.zeros((128, NPASS, 4 * HC), np.float32)
    for p in range(3):  # T1: h(dy,0) | h(dy,1)
        wpk[0:64, p] = whc[:, p, 0]
        wpk[64:128, p] = whc[:, p, 1]
    for p in range(3):  # T2: h(dy,2) | x(dy,0) | x(dy,1)
        wpk[0:64, 3 + p] = whc[:, p, 2]
        wpk[64:96, 3 + p] = wxc[:, p, 0]
        wpk[96:128, 3 + p] = wxc[:, p, 1]
    # T3: x(0,2) | x(1,2) | x(2,2) | ones row carrying the gate biases
    wpk[0:32, 6] = wxc[:, 0, 2]
    wpk[32:64, 6] = wxc[:, 1, 2]
    wpk[64:96, 6] = wxc[:, 2, 2]
    bf = np.asarray(inputs["b_xf"], np.float32) + np.asarray(inputs["b_hf"], np.float32)
    bi_ = np.asarray(inputs["b_xi"], np.float32) + np.asarray(inputs["b_hi"], np.float32)
    bo = np.asarray(inputs["b_xo"], np.float32) + np.asarray(inputs["b_ho"], np.float32)
    bg = np.asarray(inputs["b_xg"], np.float32) + np.asarray(inputs["b_hg"], np.float32)
    wpk[96, 6] = np.concatenate([bf, bi_, bo, bg])
    wcat = wpk.reshape(128, NPASS * 4 * HC).astype(BF16)

    # padded image [B, 96, HP+2, WP], h channels first; extra rows/cols give
    # room for the baked shifts below
    HPP, WPP = HP + 2, W + 2
    img = np.zeros((B, CTOT, HPP, WPP), np.float32)
    img[:, 0:HC, 1 : 1 + H, 1 : 1 + W] = h
    img[:, HC:CTOT, 1 : 1 + H, 1 : 1 + W] = x

    def shift(c0, c1, sy, sx):
        return img[:, c0:c1, sy : sy + HP, sx : sx + W]

    # bake shifts so each block's tile loads are a single contiguous window
    t1 = np.concatenate([shift(0, HC, 0, 0), shift(0, HC, 0, 1)], 1).astype(BF16)
    t2 = np.concatenate(
        [shift(0, HC, 0, 2), shift(HC, CTOT, 0, 0), shift(HC, CTOT, 0, 1)], 1
    ).astype(BF16)
    ones = np.ones((B, 1, HP, W), np.float32)
    t3 = np.concatenate(
        [shift(HC, CTOT, 0, 2), shift(HC, CTOT, 1, 2), shift(HC, CTOT, 2, 2), ones], 1
    ).astype(BF16)

    in_maps = []
    for i in range(N_CORES):
        s = slice(i * B_LOC, (i + 1) * B_LOC)
        in_maps.append(
            {
                "t1": np.ascontiguousarray(t1[s]),
                "t2": np.ascontiguousarray(t2[s]),
                "t3": np.ascontiguousarray(t3[s]),
                "c": c[s],
                "w": wcat,
            }
        )
    return in_maps


def run(inputs, trace=False, trace_kwargs=None):
    from concourse.bass_utils import run_bass_kernel_spmd

    nc = get_program()
    in_maps = _prep_host(inputs)
    res = run_bass_kernel_spmd(
        nc,
        in_maps,
        list(range(N_CORES)),
        trace=trace,
        **(trace_kwargs or {}),
    )
    h_new = np.concatenate([r["out"][:, 0] for r in res.results], 0).astype(
        np.float32
    )
    c_new = np.concatenate([r["out"][:, 1] for r in res.results], 0).astype(
        np.float32
    )
    return (h_new, c_new), res


def kernel(**inputs):
    (h_new, c_new), _ = run(inputs, trace=False)
    return (h_new, c_new)
